# revision 5
# baseline (speedup 1.0000x reference)
"""Trainium2 Bass kernel for a dense transformer decoder block (encoder-style
attention, no mask).

Shapes (hardcoded): B=2, T=2048, C=1024, H=16, Dh=64, F=4*C=4096.

Sharding: sequence-parallel over 8 cores = 2 batches x 4 token chunks of 512.
Each core computes LN1+QKV for its 512 tokens, the K/V shards are AllGathered
within each 4-core batch group, then each core runs full attention for its
queries (all 16 heads), output projection, LN2 and the full FFN for its
tokens.  The only collective is one 2MB->8MB AllGather per group.

Layout: activations are kept "transposed" (features on partitions, tokens on
the free axis) throughout, which makes every matmul natural (contraction on
partitions) with zero on-chip transposes.  Matmuls run in bf16 with fp32 PSUM
accumulation; residuals and LN statistics stay fp32.  Softmax normalization is
deferred until after the A@V matmul by augmenting V with a ones column.
"""

import numpy as np
import ml_dtypes

import concourse.bass as bass
import concourse.mybir as mybir
import concourse.tile as tile
from concourse import bacc
from concourse.bass_utils import run_bass_kernel_spmd

F32 = mybir.dt.float32
BF16 = mybir.dt.bfloat16

B, T, C, H = 2, 2048, 1024, 16
Dh = C // H            # 64
F = 4 * C              # 4096
NC = 8                 # cores
GRP = 4                # cores per batch group
TLOC = T // GRP        # 512 tokens per core
P = 128                # partitions
NCT = C // P           # 8 c-tiles
NFT = F // P           # 32 f-tiles
NKT = T // P           # 16 k-tiles (full batch seq)
NPAIR = H // 2         # 8 head pairs
EPS = 1e-5
SCALE = C ** -0.5      # note: reference scales by C**-0.5, not Dh**-0.5

REPLICA_GROUPS = [[0, 1, 2, 3], [4, 5, 6, 7]]


def build_program(fake_collective=False):
    """Build the SPMD Bacc program (identical on all 8 cores).

    fake_collective=True replaces the AllGather with local DRAM copies of the
    right size (wrong data, same timing shape) so the program has no
    collectives and can run in TimelineSim for cost-model analysis.
    """
    nc = bacc.Bacc("TRN2", target_bir_lowering=False, debug=False, num_devices=NC)

    # ---- kernel I/O (per core) ----
    xT_ext = nc.dram_tensor("xT", [C, TLOC], F32, kind="ExternalInput")
    wq_ext = nc.dram_tensor("wq", [C, C], BF16, kind="ExternalInput")
    wk_ext = nc.dram_tensor("wk", [C, C], BF16, kind="ExternalInput")
    wv_ext = nc.dram_tensor("wv", [C, C], BF16, kind="ExternalInput")
    wo_ext = nc.dram_tensor("wo", [C, C], BF16, kind="ExternalInput")
    w1_ext = nc.dram_tensor("w1", [C, F], BF16, kind="ExternalInput")
    w2_ext = nc.dram_tensor("w2", [F, C], BF16, kind="ExternalInput")
    # packed per-channel vectors: [g1, be1, g2, be2, bo, b2]
    cv_ext = nc.dram_tensor("cvecs", [6, C], F32, kind="ExternalInput")
    b1_ext = nc.dram_tensor("b1v", [F], F32, kind="ExternalInput")
    out_ext = nc.dram_tensor("outT", [C, TLOC], F32, kind="ExternalOutput")

    with tile.TileContext(nc) as tc:
        with (
            tc.tile_pool(name="persist", bufs=1) as persist,
            tc.tile_pool(name="dram", bufs=1, space="DRAM") as dram,
        ):
            # ---- persistent tiles ----
            xT = persist.tile([P, NCT, TLOC], F32)        # x^T (c, t) fp32
            QT = persist.tile([P, NPAIR, TLOC], BF16)     # Q^T head-pair tiles
            x2T = persist.tile([P, NCT, TLOC], F32)       # x + attn residual
            Wo_sb = persist.tile([P, NPAIR, C], BF16)
            cvec = persist.tile([P, 6, NCT], F32)         # per-c-tile columns
            b1c = persist.tile([P, NFT], F32)              # b1 columns
            ones_c = persist.tile([P, 1], BF16)            # stat-sum lhsT
            ones_r = persist.tile([1, P], F32)             # broadcast lhsT
            eps1 = persist.tile([1, 1], F32)               # LN epsilon

            kv_in = dram.tile([16, P, TLOC], BF16)
            kv_out = dram.tile([GRP, 16, P, TLOC], BF16)

            nc.sync.dma_start(out=xT, in_=xT_ext.rearrange("(a p) t -> p a t", p=P))
            nc.sync.dma_start(out=Wo_sb, in_=wo_ext.rearrange("(a p) n -> p a n", p=P))
            nc.gpsimd.dma_start(out=cvec, in_=cv_ext.rearrange("v (a p) -> p v a", p=P))
            nc.gpsimd.dma_start(out=b1c, in_=b1_ext.rearrange("(a p) -> p a", p=P))
            nc.vector.memset(ones_c, 1.0)
            nc.vector.memset(ones_r, 1.0)
            nc.vector.memset(eps1, EPS)

            # ================= Phase A: LN1 + QKV + AllGather(K,V) ============
            with (
                tc.tile_pool(name="phA", bufs=1) as phA,
                tc.tile_pool(name="phA_s", bufs=3) as phA_s,
                tc.tile_pool(name="psA_stat", bufs=2, space="PSUM") as psA_stat,
                tc.tile_pool(name="psA_bc", bufs=2, space="PSUM") as psA_bc,
                tc.tile_pool(name="psA_mm", bufs=4, space="PSUM") as psA_mm,
            ):
                Wq_sb = phA.tile([P, NCT, C], BF16)
                Wk_sb = phA.tile([P, NCT, C], BF16)
                Wv_sb = phA.tile([P, NCT, C], BF16)
                nc.sync.dma_start(out=Wk_sb, in_=wk_ext.rearrange("(a p) n -> p a n", p=P))
                nc.sync.dma_start(out=Wv_sb, in_=wv_ext.rearrange("(a p) n -> p a n", p=P))
                nc.sync.dma_start(out=Wq_sb, in_=wq_ext.rearrange("(a p) n -> p a n", p=P))

                hT = phA.tile([P, NCT, TLOC], BF16)
                KTl = phA.tile([P, NPAIR, TLOC], BF16)
                Vl = phA.tile([P, GRP, 2 * TLOC], BF16)  # (t-part, [tt], d)

                # --- LN1 stats: sum / sumsq rows via ones-matmul ---
                sum_ps = psA_stat.tile([1, TLOC], F32, tag="stat")
                ssq_ps = psA_stat.tile([1, TLOC], F32, tag="stat")
                for ct in range(NCT):
                    xbf = phA_s.tile([P, TLOC], BF16, tag="xbf")
                    nc.any.tensor_copy(xbf, xT[:, ct, :])
                    nc.tensor.matmul(sum_ps, ones_c, xbf,
                                     start=(ct == 0), stop=(ct == NCT - 1))
                    sq = phA_s.tile([P, TLOC], BF16, tag="sq")
                    nc.vector.tensor_mul(sq, xbf, xbf)
                    nc.tensor.matmul(ssq_ps, ones_c, sq,
                                     start=(ct == 0), stop=(ct == NCT - 1))

                mu_r = phA.tile([1, TLOC], F32)
                rstd_r = phA.tile([1, TLOC], F32)
                var_r = phA.tile([1, TLOC], F32)
                nc.scalar.activation(mu_r, sum_ps, mybir.ActivationFunctionType.Copy,
                                     scale=1.0 / C)
                musq = phA.tile([1, TLOC], F32)
                nc.vector.tensor_mul(musq, mu_r, mu_r)
                nc.vector.scalar_tensor_tensor(var_r, ssq_ps, 1.0 / C, musq,
                                               mybir.AluOpType.mult,
                                               mybir.AluOpType.subtract)
                nc.scalar.activation(var_r, var_r, mybir.ActivationFunctionType.Sqrt,
                                     bias=eps1[:])
                nc.vector.reciprocal(rstd_r, var_r)

                # broadcast stat rows to 128 partitions (fp32 matmul with ones)
                mu_ps = psA_bc.tile([P, TLOC], F32, tag="bc")
                rstd_ps = psA_bc.tile([P, TLOC], F32, tag="bc")
                nc.tensor.matmul(mu_ps, ones_r, mu_r, start=True, stop=True)
                nc.tensor.matmul(rstd_ps, ones_r, rstd_r, start=True, stop=True)
                mu_b = phA.tile([P, TLOC], F32)
                rstd_b = phA.tile([P, TLOC], F32)
                nc.any.tensor_copy(mu_b, mu_ps)
                nc.any.tensor_copy(rstd_b, rstd_ps)

                # h^T = ((x^T - mu) * g) * rstd + be   (g,be per-partition)
                for ct in range(NCT):
                    t1 = phA_s.tile([P, TLOC], F32, tag="t1")
                    nc.vector.tensor_sub(t1, xT[:, ct, :], mu_b)
                    t2 = phA_s.tile([P, TLOC], F32, tag="t2")
                    nc.vector.scalar_tensor_tensor(
                        t2, t1, cvec[:, 0, ct:ct + 1], rstd_b,
                        mybir.AluOpType.mult, mybir.AluOpType.mult)
                    nc.vector.tensor_scalar(
                        hT[:, ct, :], t2, cvec[:, 1, ct:ct + 1], None,
                        mybir.AluOpType.add)

                # --- K^T then V then AllGather, then Q^T (overlaps the AG) ---
                for pair in range(NPAIR):
                    kp = psA_mm.tile([P, TLOC], F32, tag="mm")
                    cols = slice(pair * P, (pair + 1) * P)
                    for ct in range(NCT):
                        nc.tensor.matmul(kp, Wk_sb[:, ct, cols], hT[:, ct, :],
                                         start=(ct == 0), stop=(ct == NCT - 1))
                    nc.any.tensor_copy(KTl[:, pair, :], kp)

                for tt in range(GRP):
                    for dc in range(2):
                        vp = psA_mm.tile([P, TLOC], F32, tag="mm")
                        dcol = slice(dc * TLOC, (dc + 1) * TLOC)
                        for ct in range(NCT):
                            nc.tensor.matmul(
                                vp, hT[:, ct, tt * P:(tt + 1) * P],
                                Wv_sb[:, ct, dcol],
                                start=(ct == 0), stop=(ct == NCT - 1))
                        nc.any.tensor_copy(Vl[:, tt, dcol], vp)

                nc.gpsimd.dma_start(
                    out=kv_in[0:NPAIR].rearrange("s p t -> p s t"), in_=KTl)
                nc.gpsimd.dma_start(
                    out=kv_in[NPAIR:16].rearrange("s p t -> p s t"),
                    in_=Vl.rearrange("p a (b t) -> p (a b) t", b=2))
                if fake_collective:
                    for r in range(GRP):
                        nc.gpsimd.dma_start(out=kv_out[r], in_=kv_in[:])
                else:
                    nc.gpsimd.collective_compute(
                        "AllGather", mybir.AluOpType.bypass,
                        replica_groups=REPLICA_GROUPS,
                        ins=[kv_in.opt()], outs=[kv_out.opt()])

                for pair in range(NPAIR):
                    qp = psA_mm.tile([P, TLOC], F32, tag="mm")
                    cols = slice(pair * P, (pair + 1) * P)
                    for ct in range(NCT):
                        nc.tensor.matmul(qp, Wq_sb[:, ct, cols], hT[:, ct, :],
                                         start=(ct == 0), stop=(ct == NCT - 1))
                    nc.any.tensor_copy(QT[:, pair, :], qp)

            # ================= Phase B: attention =============================
            with (
                tc.tile_pool(name="phB", bufs=1) as phB,
                tc.tile_pool(name="phB_exp", bufs=4) as phB_exp,
                tc.tile_pool(name="phB_rc", bufs=2) as phB_rc,
                tc.tile_pool(name="psB_sc", bufs=4, space="PSUM") as psB_sc,
                tc.tile_pool(name="psB_av", bufs=2, space="PSUM") as psB_av,
                tc.tile_pool(name="psB_rb", bufs=2, space="PSUM") as psB_rb,
            ):
                KT = phB.tile([P, NPAIR, T], BF16)
                Vaug = phB.tile([P, NKT, H, Dh + 1], BF16)
                OT = phB.tile([P, NPAIR, TLOC], BF16)

                for kt in range(NKT):
                    nc.vector.memset(Vaug[:, kt, :, Dh:Dh + 1], 1.0)
                for pair in range(NPAIR):
                    for r in range(GRP):
                        nc.sync.dma_start(
                            out=KT[:, pair, r * TLOC:(r + 1) * TLOC],
                            in_=kv_out[r, pair])
                for kt in range(NKT):
                    r, tt = kt // GRP, kt % GRP
                    for dc in range(2):
                        nc.sync.dma_start(
                            out=Vaug[:, kt, dc * 8:(dc + 1) * 8, 0:Dh],
                            in_=kv_out[r, NPAIR + tt * 2 + dc].rearrange(
                                "p (h d) -> p h d", d=Dh))

                for pair in range(NPAIR):
                    hA, hB = 2 * pair, 2 * pair + 1
                    oA = psB_av.tile([Dh + 1, TLOC], F32, tag="av")
                    oB = psB_av.tile([Dh + 1, TLOC], F32, tag="av")
                    for kt in range(NKT):
                        ks = slice(kt * P, (kt + 1) * P)
                        sA = psB_sc.tile([P, TLOC], F32, tag="sc")
                        sB = psB_sc.tile([P, TLOC], F32, tag="sc")
                        nc.tensor.matmul(sA, KT[0:64, pair, ks], QT[0:64, pair, :],
                                         start=True, stop=True,
                                         tile_position=(0, 0))
                        nc.tensor.matmul(sB, KT[64:128, pair, ks], QT[64:128, pair, :],
                                         start=True, stop=True,
                                         tile_position=(64, 0))
                        eA = phB_exp.tile([P, TLOC], BF16, tag="exp")
                        eB = phB_exp.tile([P, TLOC], BF16, tag="exp")
                        nc.scalar.activation(eA, sA, mybir.ActivationFunctionType.Exp,
                                             scale=SCALE)
                        nc.scalar.activation(eB, sB, mybir.ActivationFunctionType.Exp,
                                             scale=SCALE)
                        nc.tensor.matmul(oA, Vaug[:, kt, hA, :], eA,
                                         start=(kt == 0), stop=(kt == NKT - 1))
                        nc.tensor.matmul(oB, Vaug[:, kt, hB, :], eB,
                                         start=(kt == 0), stop=(kt == NKT - 1))
                    # normalize: OT = o[:64] * (1 / denom-row), broadcast via PE
                    for half, ops in ((0, oA), (1, oB)):
                        rcp = phB_rc.tile([1, TLOC], F32, tag="rcp")
                        nc.vector.reciprocal(rcp, ops[Dh:Dh + 1, :])
                        rb_ps = psB_rb.tile([Dh, TLOC], F32, tag="rb")
                        nc.tensor.matmul(rb_ps, ones_r[:, 0:Dh], rcp,
                                         start=True, stop=True)
                        rb = phB_rc.tile([Dh, TLOC], F32, tag="rb_sb")
                        nc.any.tensor_copy(rb, rb_ps)
                        nc.vector.tensor_mul(
                            OT[half * Dh:(half + 1) * Dh, pair, :],
                            ops[0:Dh, :], rb)

                # attn^T (c,t) + residual: x2T = (attn + bo) + xT
                for cc in range(NCT):
                    ap = psB_sc.tile([P, TLOC], F32, tag="sc")
                    for pair in range(NPAIR):
                        nc.tensor.matmul(ap, Wo_sb[:, pair, cc * P:(cc + 1) * P],
                                         OT[:, pair, :],
                                         start=(pair == 0), stop=(pair == NPAIR - 1))
                    nc.vector.scalar_tensor_tensor(
                        x2T[:, cc, :], ap, cvec[:, 4, cc:cc + 1], xT[:, cc, :],
                        mybir.AluOpType.add, mybir.AluOpType.add)

            # ================= Phase C: LN2 + FFN =============================
            with (
                tc.tile_pool(name="phC", bufs=1) as phC,
                tc.tile_pool(name="phC_s", bufs=2) as phC_s,
                tc.tile_pool(name="phC_o", bufs=2) as phC_o,
                tc.tile_pool(name="phC_w1", bufs=3) as phC_w1,
                tc.tile_pool(name="psC_stat", bufs=2, space="PSUM") as psC_stat,
                tc.tile_pool(name="psC_bc", bufs=2, space="PSUM") as psC_bc,
                tc.tile_pool(name="psC_mm", bufs=4, space="PSUM") as psC_mm,
            ):
                W2_sb = phC.tile([P, NFT, C], BF16)
                nc.sync.dma_start(out=W2_sb, in_=w2_ext.rearrange("(a p) n -> p a n", p=P))
                h2T = phC.tile([P, NCT, TLOC], BF16)
                relu = phC.tile([P, NFT, TLOC], BF16)

                sum_ps = psC_stat.tile([1, TLOC], F32, tag="stat")
                ssq_ps = psC_stat.tile([1, TLOC], F32, tag="stat")
                for ct in range(NCT):
                    xbf = phC_s.tile([P, TLOC], BF16, tag="xbf")
                    nc.any.tensor_copy(xbf, x2T[:, ct, :])
                    nc.tensor.matmul(sum_ps, ones_c, xbf,
                                     start=(ct == 0), stop=(ct == NCT - 1))
                    sq = phC_s.tile([P, TLOC], BF16, tag="sq")
                    nc.vector.tensor_mul(sq, xbf, xbf)
                    nc.tensor.matmul(ssq_ps, ones_c, sq,
                                     start=(ct == 0), stop=(ct == NCT - 1))

                mu_r = phC.tile([1, TLOC], F32)
                rstd_r = phC.tile([1, TLOC], F32)
                var_r = phC.tile([1, TLOC], F32)
                nc.scalar.activation(mu_r, sum_ps, mybir.ActivationFunctionType.Copy,
                                     scale=1.0 / C)
                musq = phC.tile([1, TLOC], F32)
                nc.vector.tensor_mul(musq, mu_r, mu_r)
                nc.vector.scalar_tensor_tensor(var_r, ssq_ps, 1.0 / C, musq,
                                               mybir.AluOpType.mult,
                                               mybir.AluOpType.subtract)
                nc.scalar.activation(var_r, var_r, mybir.ActivationFunctionType.Sqrt,
                                     bias=eps1[:])
                nc.vector.reciprocal(rstd_r, var_r)
                mu_ps = psC_bc.tile([P, TLOC], F32, tag="bc")
                rstd_ps = psC_bc.tile([P, TLOC], F32, tag="bc")
                nc.tensor.matmul(mu_ps, ones_r, mu_r, start=True, stop=True)
                nc.tensor.matmul(rstd_ps, ones_r, rstd_r, start=True, stop=True)
                mu_b = phC.tile([P, TLOC], F32)
                rstd_b = phC.tile([P, TLOC], F32)
                nc.any.tensor_copy(mu_b, mu_ps)
                nc.any.tensor_copy(rstd_b, rstd_ps)

                for ct in range(NCT):
                    t1 = phC_s.tile([P, TLOC], F32, tag="t1")
                    nc.vector.tensor_sub(t1, x2T[:, ct, :], mu_b)
                    t2 = phC_s.tile([P, TLOC], F32, tag="t2")
                    nc.vector.scalar_tensor_tensor(
                        t2, t1, cvec[:, 2, ct:ct + 1], rstd_b,
                        mybir.AluOpType.mult, mybir.AluOpType.mult)
                    nc.vector.tensor_scalar(
                        h2T[:, ct, :], t2, cvec[:, 3, ct:ct + 1], None,
                        mybir.AluOpType.add)

                # FFN1: relu = relu(W1^T @ h2 + b1), (f, t) layout
                w1r = w1_ext.rearrange("(a p) n -> p a n", p=P)
                for ft in range(NFT):
                    w1t = phC_w1.tile([P, NCT, P], BF16, tag="w1")
                    nc.sync.dma_start(out=w1t, in_=w1r[:, :, ft * P:(ft + 1) * P])
                    fp = psC_mm.tile([P, TLOC], F32, tag="mm")
                    for ct in range(NCT):
                        nc.tensor.matmul(fp, w1t[:, ct, :], h2T[:, ct, :],
                                         start=(ct == 0), stop=(ct == NCT - 1))
                    nc.scalar.activation(relu[:, ft, :], fp,
                                         mybir.ActivationFunctionType.Relu,
                                         bias=b1c[:, ft:ft + 1])

                # FFN2 + residual: out = (W2^T @ relu + b2) + x2
                outr = out_ext.rearrange("(a p) t -> p a t", p=P)
                for cc in range(NCT):
                    op = psC_mm.tile([P, TLOC], F32, tag="mm")
                    for ft in range(NFT):
                        nc.tensor.matmul(op, W2_sb[:, ft, cc * P:(cc + 1) * P],
                                         relu[:, ft, :],
                                         start=(ft == 0), stop=(ft == NFT - 1))
                    out_t = phC_o.tile([P, TLOC], F32, tag="out")
                    nc.vector.scalar_tensor_tensor(
                        out_t, op, cvec[:, 5, cc:cc + 1], x2T[:, cc, :],
                        mybir.AluOpType.add, mybir.AluOpType.add)
                    nc.sync.dma_start(out=outr[:, cc, :], in_=out_t)

    nc.compile()
    return nc


_NC_CACHE = None


def get_program():
    global _NC_CACHE
    if _NC_CACHE is None:
        _NC_CACHE = build_program()
    return _NC_CACHE


def make_in_maps(x, Wq, Wk, Wv, Wo, bo, W1, b1, W2, b2, g1, be1, g2, be2):
    """Host-side sharding / layout prep. Returns per-core input dicts."""
    bf = ml_dtypes.bfloat16
    x = np.asarray(x, np.float32)
    wq = np.ascontiguousarray(
        np.transpose(np.asarray(Wq, np.float32), (1, 0, 2)).reshape(C, C)).astype(bf)
    wk = np.ascontiguousarray(
        np.transpose(np.asarray(Wk, np.float32), (1, 0, 2)).reshape(C, C)).astype(bf)
    wv = np.ascontiguousarray(
        np.transpose(np.asarray(Wv, np.float32), (1, 0, 2)).reshape(C, C)).astype(bf)
    wo = np.asarray(Wo, np.float32).astype(bf)
    w1 = np.asarray(W1, np.float32).astype(bf)
    w2 = np.asarray(W2, np.float32).astype(bf)
    cvecs = np.stack([
        np.asarray(g1, np.float32), np.asarray(be1, np.float32),
        np.asarray(g2, np.float32), np.asarray(be2, np.float32),
        np.asarray(bo, np.float32), np.asarray(b2, np.float32)]).astype(np.float32)
    b1v = np.asarray(b1, np.float32)

    in_maps = []
    for i in range(NC):
        b, j = i // GRP, i % GRP
        xT = np.ascontiguousarray(x[b, j * TLOC:(j + 1) * TLOC, :].T)
        in_maps.append({
            "xT": xT, "wq": wq, "wk": wk, "wv": wv, "wo": wo,
            "w1": w1, "w2": w2, "cvecs": cvecs, "b1v": b1v,
        })
    return in_maps


def assemble_output(results):
    """results: list of per-core dicts with 'outT' (C, TLOC) fp32."""
    out = np.empty((B, T, C), np.float32)
    for i in range(NC):
        b, j = i // GRP, i % GRP
        out[b, j * TLOC:(j + 1) * TLOC, :] = results[i]["outT"].T
    return out


def kernel(**inputs) -> np.ndarray:
    nc = get_program()
    in_maps = make_in_maps(**inputs)
    res = run_bass_kernel_spmd(nc, in_maps, core_ids=list(range(NC)))
    return assemble_output(res.results)
